# revision 35
# baseline (speedup 1.0000x reference)
# Bass/Trainium2 kernel for MHConvAttention (B=16, C=128, H=W=64, NH=8, OUT=512)
# Data-parallel over batch: 8 cores x 2 samples each.
#
# Per-sample layout: channels (128) on SBUF partitions, flattened spatial (4096)
# on the free dim. Depthwise convs are diagonal-weight matmuls accumulated in
# PSUM over zero-padded SBUF buffers; the 5x5 rel-pos conv runs as fp8
# DoubleRow tap-pairs (2 taps per matmul); the content-lambda path uses a
# transposed QKV GEMM (spatial-on-partitions) so no explicit transposes are
# needed; the ECA channel-attention is folded into the out-projection weights.
# The out-projection is software-pipelined into the conv chunk loop and its
# results DMA straight from PSUM to DRAM, spread across engine queues.
import os
import numpy as np

B, C, H, W = 16, 128, 64, 64
NH, HD, WIN, OUT = 8, 16, 5, 512
N = H * W
NCORES = 8
SPC = B // NCORES          # samples per core
NC8 = N // 512             # 512-wide chunks per sample
NJ = N // 128              # 128-wide chunks (transposed GEMM)
SCALING = HD ** (-0.5)
PLS = 64.0                 # fp8 pre-scale for the 5x5 rel-pos weights

_CACHE = {}


def _build_nc():
    import concourse.bass as bass
    import concourse.tile as tile
    import concourse.mybir as mybir
    from concourse import bacc

    f32 = mybir.dt.float32
    f32r = mybir.dt.float32r
    bf16 = mybir.dt.bfloat16
    fp8 = mybir.dt.float8e4
    DR = mybir.MatmulPerfMode.DoubleRow
    Alu = mybir.AluOpType
    Act = mybir.ActivationFunctionType

    def r(ap):
        return ap.bitcast(f32r)

    nc = bacc.Bacc(trn_type="TRN2", target_bir_lowering=False, debug=False)

    src_d = nc.dram_tensor("src", [SPC, C, H + 2, W + 2], bf16, kind="ExternalInput").ap()
    d3_d = nc.dram_tensor("d3", [C, 9, C], bf16, kind="ExternalInput").ap()
    d5_d = nc.dram_tensor("d5", [C, 13, 2, C], fp8, kind="ExternalInput").ap()
    wq_d = nc.dram_tensor("wq", [C, C], f32, kind="ExternalInput").ap()
    wv_d = nc.dram_tensor("wv", [C, C], f32, kind="ExternalInput").ap()
    wkv_d = nc.dram_tensor("wkv", [C, 2 * C], f32, kind="ExternalInput").ap()
    w1_d = nc.dram_tensor("w1", [C, OUT], f32, kind="ExternalInput").ap()
    w2_d = nc.dram_tensor("w2", [C, OUT], f32, kind="ExternalInput").ap()
    mask_d = nc.dram_tensor("mask", [C, C], f32, kind="ExternalInput").ap()
    trid_d = nc.dram_tensor("trid", [C, C], f32, kind="ExternalInput").ap()
    out_d = nc.dram_tensor("out", [SPC, OUT, H, W], bf16, kind="ExternalOutput").ap()
    out_v = out_d.rearrange("s o h w -> s o (h w)")

    with tile.TileContext(nc) as tc, __import__("contextlib").ExitStack() as ctx:
        wpool = ctx.enter_context(tc.tile_pool(name="w", bufs=1))
        srcp_pool = ctx.enter_context(tc.tile_pool(name="srcp", bufs=2))
        s_pool = ctx.enter_context(tc.tile_pool(name="s", bufs=8))
        q_pool = ctx.enter_context(tc.tile_pool(name="q", bufs=16))
        r1_pool = ctx.enter_context(tc.tile_pool(name="r1", bufs=3))
        vpad_pool = ctx.enter_context(tc.tile_pool(name="vpad", bufs=2))
        eT_pool = ctx.enter_context(tc.tile_pool(name="eT", bufs=2))
        vT_pool = ctx.enter_context(tc.tile_pool(name="vT", bufs=2))
        tmp_pool = ctx.enter_context(tc.tile_pool(name="tmp", bufs=2))
        w2p_pool = ctx.enter_context(tc.tile_pool(name="w2p", bufs=2))
        stage_pool = ctx.enter_context(tc.tile_pool(name="stage", bufs=4))
        cln_pool = ctx.enter_context(tc.tile_pool(name="cln", bufs=2))
        small_pool = ctx.enter_context(tc.tile_pool(name="small", bufs=4))
        ps_pool = ctx.enter_context(tc.tile_pool(name="ps", bufs=8, space="PSUM"))

        # ---- prologue DMAs: weights on the scalar queue (in order of first
        # use), per-sample src slices on sync/gpsimd queues ----
        d3_sb = wpool.tile([C, 9, C], bf16)
        for sl in range(3):
            nc.scalar.dma_start(
                d3_sb[:, 3 * sl : 3 * sl + 3, :], d3_d[:, 3 * sl : 3 * sl + 3, :]
            )
        srcp_t = []
        rsplit = [0, 10, 18, 26, 34, 42, 50, 58, 66]
        for smp in range(SPC):
            srcp = srcp_pool.tile([C, H + 2, W + 2], bf16, tag="srcp")
            for k in range(8):
                lo, hi = rsplit[k], rsplit[k + 1]
                eng = nc.sync if smp == 0 else nc.gpsimd
                eng.dma_start(
                    srcp[:, lo:hi, :],
                    src_d[smp, :, lo:hi, :],
                )
            srcp_t.append(srcp)
        wq_sb = wpool.tile([C, C], f32)
        nc.scalar.dma_start(r(wq_sb[:]), r(wq_d[:]))
        wv_sb = wpool.tile([C, C], f32)
        nc.scalar.dma_start(r(wv_sb[:]), r(wv_d[:]))
        wkv_sb = wpool.tile([C, 2 * C], f32)
        nc.scalar.dma_start(r(wkv_sb[:]), r(wkv_d[:]))
        d5_sb = wpool.tile([C, 13, 2, C], fp8)
        nc.scalar.dma_start(d5_sb[:], d5_d[:])
        trid_sb = wpool.tile([C, C], f32)
        nc.scalar.dma_start(trid_sb[:], trid_d[:])
        mask_sb = wpool.tile([C, C], f32)
        nc.scalar.dma_start(mask_sb[:], mask_d[:])
        w1_sb = wpool.tile([C, OUT], f32)
        nc.scalar.dma_start(r(w1_sb[:]), r(w1_d[:]))
        w2_sb = wpool.tile([C, OUT], f32)
        nc.scalar.dma_start(w2_sb[:], w2_d[:])

        WROW = W + 4
        state = []
        # ---- pass 1: both samples' CPE / forward QKV / transposed KV GEMMs.
        # Doing all prep first lets each sample's out-projection DMA stream
        # start earlier and spread over a longer window. ----
        for smp in range(SPC):
            srcp = srcp_t[smp]

            # ---- ECA pooling: 4 partial reduces interleaved with QKV copies ----
            pool_part = small_pool.tile([C, 4], f32, tag="pool_part")
            pool_sum = small_pool.tile([C, 1], f32, tag="psum_vec")

            # ---- CPE 3x3 depthwise conv (+residual, folded): s = conv3(src) ----
            s_t = []
            for c8 in range(NC8):
                ps = ps_pool.tile([C, 512], f32, tag="ps")
                y0 = 8 * c8
                for tap in range(9):
                    dy, dx = tap // 3, tap % 3
                    nc.tensor.matmul(
                        ps[:],
                        d3_sb[:, tap, :],
                        srcp[:, y0 + dy : y0 + dy + 8, dx : dx + W],
                        start=(tap == 0),
                        stop=(tap == 8),
                    )
                st = s_pool.tile([C, 512], f32, tag="s")
                if c8 % 2 == 0:
                    nc.vector.tensor_copy(r(st[:]), ps[:])
                else:
                    nc.scalar.copy(r(st[:]), ps[:])
                s_t.append(st)

            # ---- forward QKV GEMM: q and v in channels-on-partitions layout ----
            vpad = vpad_pool.tile([C, H + 4, W + 4], fp8, tag="vpad")
            nc.gpsimd.memset(vpad[:, 0:2, :], 0.0)
            nc.gpsimd.memset(vpad[:, H + 2 : H + 4, :], 0.0)
            nc.gpsimd.memset(vpad[:, :, 0:2], 0.0)
            nc.gpsimd.memset(vpad[:, :, W + 2 : W + 4], 0.0)
            q_t = []
            for c8 in range(NC8):
                psq = ps_pool.tile([C, 512], f32, tag="ps")
                nc.tensor.matmul(psq[:], r(wq_sb[:]), r(s_t[c8][:]), start=True, stop=True)
                qt = q_pool.tile([C, 512], f32, tag="q")
                qeng = nc.scalar if c8 % 2 == 0 else nc.vector
                veng = nc.vector if c8 % 2 == 0 else nc.scalar
                if qeng is nc.scalar:
                    nc.scalar.copy(r(qt[:]), psq[:])
                else:
                    nc.vector.tensor_copy(r(qt[:]), psq[:])
                q_t.append(qt)
                psv = ps_pool.tile([C, 512], f32, tag="ps")
                nc.tensor.matmul(psv[:], r(wv_sb[:]), r(s_t[c8][:]), start=True, stop=True)
                vdst = vpad[:, 2 + 8 * c8 : 2 + 8 * c8 + 8, 2 : W + 2]
                vsrc = psv[:].rearrange("p (a b) -> p a b", a=8)
                if veng is nc.scalar:
                    nc.scalar.copy(vdst, vsrc)
                else:
                    nc.vector.tensor_copy(vdst, vsrc)
                if c8 % 2 == 1:
                    k = c8 // 2
                    nc.vector.reduce_sum(
                        pool_part[:, k : k + 1],
                        srcp[:, 1 + 16 * k : 1 + 16 * (k + 1), 1 : W + 1],
                        axis=mybir.AxisListType.XY,
                    )
                    if c8 == NC8 - 1:
                        nc.vector.reduce_sum(
                            pool_sum[:], pool_part[:], axis=mybir.AxisListType.X
                        )

            # ---- transposed GEMM: [kT | vT] chunks; exp(kT) -> eT, vT + ones col ----
            eT = eT_pool.tile([C, NJ, C], bf16, tag="eT")
            vT = vT_pool.tile([C, NJ, C + 1], bf16, tag="vT")
            nc.gpsimd.memset(vT[:, :, C : C + 1], 1.0)
            for j in range(NJ):
                psT = ps_pool.tile([C, 512], f32, tag="ps", name="psT")
                lhs = s_t[j // 4][:, (j % 4) * 128 : (j % 4 + 1) * 128]
                nc.tensor.matmul(
                    psT[:, 0 : 2 * C], r(lhs), r(wkv_sb[:]), start=True, stop=True
                )
                nc.scalar.activation(eT[:, j, :], psT[:, 0:C], Act.Exp)
                nc.vector.tensor_copy(vT[:, j, 0:C], psT[:, C : 2 * C])

            state.append(dict(q_t=q_t, vpad=vpad, eT=eT, vT=vT, pool_sum=pool_sum))

        # ---- pass 2: per-sample fused conv/content/out-proj pipeline. ----
        for smp in range(SPC):
            srcp = srcp_t[smp]
            q_t = state[smp]["q_t"]
            vpad = state[smp]["vpad"]
            eT = state[smp]["eT"]
            vT = state[smp]["vT"]
            pool_sum = state[smp]["pool_sum"]

            # ---- fused conv/content/out-proj pipeline over chunks.
            # ps5 runs one chunk ahead; the CL block is emitted after ps5(0)
            # so its eT/vT copy drain hides under conv matmuls; out-proj of
            # chunk c8 runs while conv of chunk c8+2 streams. Out-proj stages
            # one [C, 4, 512] tile per chunk and DMAs it in one transfer,
            # rotating engine queues; the last chunk splits across 4 queues
            # to keep the kernel tail short. ----
            def conv5(c8):
                ps5 = ps_pool.tile([C, 512], f32, tag="ps")
                y0 = 8 * c8
                for j in range(13):
                    ta, tb = 2 * j, min(2 * j + 1, 24)
                    dya, dxa = ta // 5, ta % 5
                    dyb, dxb = tb // 5, tb % 5
                    delta = (dyb - dya) * WROW + (dxb - dxa)
                    base = vpad[:, y0 + dya : y0 + dya + 8, dxa : dxa + W]
                    rhs = bass.AP(
                        base.tensor,
                        base.offset,
                        [list(base.ap[0]), [delta, 2], [WROW, 8], [1, W]],
                    )
                    nc.tensor.matmul(
                        ps5[:], d5_sb[:, j, :, :], rhs,
                        start=(j == 0), stop=(j == 12), perf_mode=DR,
                    )
                return ps5

            def content(c8, ps5):
                psc = ps_pool.tile([C, 512], f32, tag="ps")
                nc.tensor.matmul(psc[:], r(cln[0]), r(q_t[c8][:]), start=True, stop=True)
                tmp = tmp_pool.tile([C, 512], f32, tag="tmp")
                nc.vector.tensor_tensor(tmp[:], q_t[c8][:], ps5[:], Alu.mult)
                rt = r1_pool.tile([C, 512], f32, tag="r1")
                nc.vector.tensor_tensor(r(rt[:]), tmp[:], psc[:], Alu.add)
                return rt

            def outproj(c8, rt):
                y0 = 8 * c8
                # the last sample's final two chunks split per-m across all
                # three DMA queues to keep the kernel tail short
                split = smp == SPC - 1 and c8 >= NC8 - 2
                stg = stage_pool.tile([C, 4, 512], bf16, tag="stage")
                for m in range(OUT // C):
                    pso = ps_pool.tile([C, 512], f32, tag="ps")
                    nc.tensor.matmul(
                        pso[:], r(w1_sb[:, m * C : (m + 1) * C]), r(rt[:]),
                        start=True, stop=False,
                    )
                    nc.tensor.matmul(
                        pso[:], w2p[0][:, m * C : (m + 1) * C],
                        srcp[:, 1 + y0 : 1 + y0 + 8, 1 : W + 1],
                        start=False, stop=True,
                    )
                    if m % 2 == 0:
                        nc.scalar.copy(stg[:, m, :], pso[:])
                    else:
                        nc.vector.tensor_copy(stg[:, m, :], pso[:])
                    if split:
                        eng = (nc.gpsimd, nc.sync, nc.scalar)[(4 * c8 + m) % 3]
                        eng.dma_start(
                            out_v[smp, m * C : (m + 1) * C, c8 * 512 : (c8 + 1) * 512],
                            stg[:, m, :],
                        )
                if not split:
                    for h in range(2):
                        dst = bass.AP(
                            out_v.tensor,
                            out_v.offset + (smp * OUT + 2 * h * C) * N + c8 * 512,
                            [[N, C], [C * N, 2], [1, 512]],
                        )
                        eng = (nc.gpsimd, nc.sync, nc.scalar)[(2 * c8 + h) % 3]
                        eng.dma_start(dst, stg[:, 2 * h : 2 * h + 2, :])

            cln = [None]
            w2p = [None]
            pend5 = {}   # c8 -> ps5 awaiting content
            pendo = {}   # c8 -> rt awaiting out-projection
            for c8 in range(NC8):
                if c8 >= 1:
                    pendo[c8 - 1] = content(c8 - 1, pend5.pop(c8 - 1))
                if c8 >= 2:
                    outproj(c8 - 2, pendo.pop(c8 - 2))
                pend5[c8] = conv5(c8)
                if c8 == 0:
                    # ECA: ca = sigmoid(tridiag @ mean_pool(src)); w2 scaled
                    ps_eca = ps_pool.tile([C, 1], f32, tag="ps", name="ps_eca")
                    nc.tensor.matmul(
                        ps_eca[:], trid_sb[:], pool_sum[:], start=True, stop=True
                    )
                    ca = small_pool.tile([C, 1], f32, tag="ca")
                    nc.scalar.activation(ca[:], ps_eca[:], Act.Sigmoid)
                    w2p[0] = w2p_pool.tile([C, OUT], bf16, tag="w2p", name="w2p")
                    nc.vector.tensor_scalar(w2p[0][:], w2_sb[:], ca[:], None, Alu.mult)
                    # content lambda: CL[i, o] (+ row sums in col 128)
                    ps_cl = ps_pool.tile([C, C + 1], f32, tag="ps", name="ps_cl")
                    for j in range(NJ):
                        nc.tensor.matmul(
                            ps_cl[:], eT[:, j, :], vT[:, j, :],
                            start=(j == 0), stop=(j == NJ - 1),
                        )
                    recip = small_pool.tile([C, 1], f32, tag="recip")
                    nc.vector.reciprocal(recip[:], ps_cl[:, C : C + 1])
                    cln_t = small_pool.tile([C, C], f32, tag="cln_t")
                    nc.vector.tensor_scalar(
                        cln_t[:], ps_cl[:, 0:C], recip[:], None, Alu.mult
                    )
                    cln[0] = cln_pool.tile([C, C], f32, tag="cln", name="cln")
                    nc.vector.tensor_tensor(r(cln[0][:]), cln_t[:], mask_sb[:], Alu.mult)
            pendo[NC8 - 1] = content(NC8 - 1, pend5.pop(NC8 - 1))
            outproj(NC8 - 2, pendo.pop(NC8 - 2))
            outproj(NC8 - 1, pendo.pop(NC8 - 1))

    nc.compile()
    return nc


def _get_nc():
    if "nc" not in _CACHE:
        _CACHE["nc"] = _build_nc()
    return _CACHE["nc"]


def _host_weights(cpe_w, qkv_w, rel_pos, conv1d_w, out_w):
    cpe_w = np.asarray(cpe_w, np.float32)
    qkv_w = np.asarray(qkv_w, np.float32)
    rel_pos = np.asarray(rel_pos, np.float32)
    conv1d_w = np.asarray(conv1d_w, np.float32)
    out_w = np.asarray(out_w, np.float32)

    import ml_dtypes
    d3 = np.zeros([C, 9, C], ml_dtypes.bfloat16)
    idx = np.arange(C)
    for tap in range(9):
        dy, dx = tap // 3, tap % 3
        d3[idx, tap, idx] = cpe_w[:, 0, dy, dx].astype(ml_dtypes.bfloat16)
    d3[idx, 4, idx] = (cpe_w[:, 0, 1, 1].astype(np.float32) + 1.0).astype(
        ml_dtypes.bfloat16
    )  # residual folded into center tap

    # 5x5 rel-pos weights as fp8 DoubleRow pairs, pre-scaled by PLS so the
    # ~0.02-magnitude weights land in fp8e4m3's normal range. The position
    # path output is then PLS x too large; compensated by mask*PLS (content
    # path, so result1 is uniformly scaled) and w1/PLS (out projection).
    import ml_dtypes
    d5 = np.zeros([C, 13, 2, C], ml_dtypes.float8_e4m3fn)
    for tap in range(25):
        dy, dx = tap // 5, tap % 5
        d5[idx, tap // 2, tap % 2, idx] = (rel_pos[idx % HD, dy, dx] * PLS).astype(
            ml_dtypes.float8_e4m3fn
        )

    wq = np.ascontiguousarray(qkv_w[0:C, :].T)
    wv = np.ascontiguousarray(qkv_w[2 * C : 3 * C, :].T)
    wkv = np.ascontiguousarray(qkv_w[C : 3 * C, :].T)
    w1 = np.ascontiguousarray(out_w[:, 0:C].T) * (1.0 / PLS)
    w2 = np.ascontiguousarray(out_w[:, C : 2 * C].T)

    mask = np.zeros([C, C], np.float32)
    for h in range(NH):
        mask[h * HD : (h + 1) * HD, h * HD : (h + 1) * HD] = SCALING * PLS

    trid = np.zeros([C, C], np.float32)
    trid[idx[:-1], idx[:-1] + 1] = conv1d_w[0]  # pool[c-1] contributes to ca[c]
    trid[idx, idx] = conv1d_w[1]
    trid[idx[1:], idx[1:] - 1] = conv1d_w[2]
    trid *= 1.0 / N
    return dict(d3=d3, d5=d5, wq=wq, wv=wv, wkv=wkv, w1=w1, w2=w2,
                mask=mask, trid=trid)


def kernel(src, cpe_w, qkv_w, rel_pos, conv1d_w, out_w):
    from concourse.bass_utils import run_bass_kernel_spmd

    import ml_dtypes
    src = np.asarray(src, np.float32)
    src = np.pad(src, ((0, 0), (0, 0), (1, 1), (1, 1))).astype(ml_dtypes.bfloat16)
    w = _host_weights(cpe_w, qkv_w, rel_pos, conv1d_w, out_w)
    nc = _get_nc()
    in_maps = [
        {"src": np.ascontiguousarray(src[i * SPC : (i + 1) * SPC]), **w}
        for i in range(NCORES)
    ]
    trace = bool(os.environ.get("BASS_TRACE"))
    res = run_bass_kernel_spmd(nc, in_maps, list(range(NCORES)), trace=trace)
    _CACHE["last_result"] = res
    out = np.concatenate(
        [np.asarray(res.results[i]["out"], np.float32) for i in range(NCORES)], axis=0
    )
    return out


# revision 36
# speedup vs baseline: 1.0034x; 1.0034x over previous
# Bass/Trainium2 kernel for MHConvAttention (B=16, C=128, H=W=64, NH=8, OUT=512)
# Data-parallel over batch: 8 cores x 2 samples each.
#
# Per-sample layout: channels (128) on SBUF partitions, flattened spatial (4096)
# on the free dim. Depthwise convs are diagonal-weight matmuls accumulated in
# PSUM over zero-padded buffers; the 5x5 rel-pos conv runs as fp8 DoubleRow
# tap-pairs (2 taps per matmul, weights pre-scaled by PLS and folded back out
# via mask/w1); the content-lambda path uses a transposed QKV GEMM (spatial on
# partitions) so no explicit transposes are needed; the ECA channel-attention
# is folded into the out-projection weights. Inputs arrive host-padded in bf16
# so every DMA is contiguous; the output is written in bf16 (cast back to f32
# on the host) to halve output DMA bytes. Both samples' prep phases run before
# the two fused conv/content/out-proj chunk pipelines so the output DMA stream
# spreads across most of the kernel, rotating over the gpsimd/sync/scalar
# queues; the last two chunks split per-m to keep the kernel tail short.
import os
import numpy as np

B, C, H, W = 16, 128, 64, 64
NH, HD, WIN, OUT = 8, 16, 5, 512
N = H * W
NCORES = 8
SPC = B // NCORES          # samples per core
NC8 = N // 512             # 512-wide chunks per sample
NJ = N // 128              # 128-wide chunks (transposed GEMM)
SCALING = HD ** (-0.5)
PLS = 64.0                 # fp8 pre-scale for the 5x5 rel-pos weights

_CACHE = {}


def _build_nc():
    import concourse.bass as bass
    import concourse.tile as tile
    import concourse.mybir as mybir
    from concourse import bacc

    f32 = mybir.dt.float32
    f32r = mybir.dt.float32r
    bf16 = mybir.dt.bfloat16
    fp8 = mybir.dt.float8e4
    DR = mybir.MatmulPerfMode.DoubleRow
    Alu = mybir.AluOpType
    Act = mybir.ActivationFunctionType

    def r(ap):
        return ap.bitcast(f32r)

    nc = bacc.Bacc(trn_type="TRN2", target_bir_lowering=False, debug=False)

    src_d = nc.dram_tensor("src", [SPC, C, H + 2, W + 2], bf16, kind="ExternalInput").ap()
    d3_d = nc.dram_tensor("d3", [C, 9, C], bf16, kind="ExternalInput").ap()
    d5_d = nc.dram_tensor("d5", [C, 13, 2, C], fp8, kind="ExternalInput").ap()
    wq_d = nc.dram_tensor("wq", [C, C], f32, kind="ExternalInput").ap()
    wv_d = nc.dram_tensor("wv", [C, C], f32, kind="ExternalInput").ap()
    wkv_d = nc.dram_tensor("wkv", [C, 2 * C], f32, kind="ExternalInput").ap()
    w1_d = nc.dram_tensor("w1", [C, OUT], f32, kind="ExternalInput").ap()
    w2_d = nc.dram_tensor("w2", [C, OUT], f32, kind="ExternalInput").ap()
    mask_d = nc.dram_tensor("mask", [C, C], f32, kind="ExternalInput").ap()
    trid_d = nc.dram_tensor("trid", [C, C], f32, kind="ExternalInput").ap()
    out_d = nc.dram_tensor("out", [SPC, OUT, H, W], bf16, kind="ExternalOutput").ap()
    out_v = out_d.rearrange("s o h w -> s o (h w)")

    with tile.TileContext(nc) as tc, __import__("contextlib").ExitStack() as ctx:
        wpool = ctx.enter_context(tc.tile_pool(name="w", bufs=1))
        srcp_pool = ctx.enter_context(tc.tile_pool(name="srcp", bufs=2))
        s_pool = ctx.enter_context(tc.tile_pool(name="s", bufs=8))
        q_pool = ctx.enter_context(tc.tile_pool(name="q", bufs=16))
        r1_pool = ctx.enter_context(tc.tile_pool(name="r1", bufs=3))
        vpad_pool = ctx.enter_context(tc.tile_pool(name="vpad", bufs=2))
        eT_pool = ctx.enter_context(tc.tile_pool(name="eT", bufs=2))
        vT_pool = ctx.enter_context(tc.tile_pool(name="vT", bufs=2))
        tmp_pool = ctx.enter_context(tc.tile_pool(name="tmp", bufs=2))
        w2p_pool = ctx.enter_context(tc.tile_pool(name="w2p", bufs=2))
        stage_pool = ctx.enter_context(tc.tile_pool(name="stage", bufs=4))
        cln_pool = ctx.enter_context(tc.tile_pool(name="cln", bufs=2))
        small_pool = ctx.enter_context(tc.tile_pool(name="small", bufs=4))
        ps_pool = ctx.enter_context(tc.tile_pool(name="ps", bufs=8, space="PSUM"))

        # ---- prologue DMAs: weights on the scalar queue (in order of first
        # use), per-sample src slices on sync/gpsimd queues ----
        d3_sb = wpool.tile([C, 9, C], bf16)
        for sl in range(3):
            nc.scalar.dma_start(
                d3_sb[:, 3 * sl : 3 * sl + 3, :], d3_d[:, 3 * sl : 3 * sl + 3, :]
            )
        srcp_t = []
        rsplit = [0, 10, 18, 26, 34, 42, 50, 58, 66]
        for smp in range(SPC):
            srcp = srcp_pool.tile([C, H + 2, W + 2], bf16, tag="srcp")
            for k in range(8):
                lo, hi = rsplit[k], rsplit[k + 1]
                eng = nc.sync if k % 2 == 0 else nc.gpsimd
                eng.dma_start(
                    srcp[:, lo:hi, :],
                    src_d[smp, :, lo:hi, :],
                )
            srcp_t.append(srcp)
        wq_sb = wpool.tile([C, C], f32)
        nc.scalar.dma_start(r(wq_sb[:]), r(wq_d[:]))
        wv_sb = wpool.tile([C, C], f32)
        nc.scalar.dma_start(r(wv_sb[:]), r(wv_d[:]))
        wkv_sb = wpool.tile([C, 2 * C], f32)
        nc.scalar.dma_start(r(wkv_sb[:]), r(wkv_d[:]))
        d5_sb = wpool.tile([C, 13, 2, C], fp8)
        nc.scalar.dma_start(d5_sb[:], d5_d[:])
        trid_sb = wpool.tile([C, C], f32)
        nc.scalar.dma_start(trid_sb[:], trid_d[:])
        mask_sb = wpool.tile([C, C], f32)
        nc.scalar.dma_start(mask_sb[:], mask_d[:])
        w1_sb = wpool.tile([C, OUT], f32)
        nc.scalar.dma_start(r(w1_sb[:]), r(w1_d[:]))
        w2_sb = wpool.tile([C, OUT], f32)
        nc.scalar.dma_start(w2_sb[:], w2_d[:])

        WROW = W + 4
        state = []
        # ---- pass 1: both samples' CPE / forward QKV / transposed KV GEMMs.
        # Doing all prep first lets each sample's out-projection DMA stream
        # start earlier and spread over a longer window. ----
        for smp in range(SPC):
            srcp = srcp_t[smp]

            # ---- ECA pooling: 4 partial reduces interleaved with QKV copies ----
            pool_part = small_pool.tile([C, 4], f32, tag="pool_part")
            pool_sum = small_pool.tile([C, 1], f32, tag="psum_vec")

            # ---- CPE 3x3 depthwise conv (+residual, folded): s = conv3(src) ----
            s_t = []
            for c8 in range(NC8):
                ps = ps_pool.tile([C, 512], f32, tag="ps")
                y0 = 8 * c8
                for tap in range(9):
                    dy, dx = tap // 3, tap % 3
                    nc.tensor.matmul(
                        ps[:],
                        d3_sb[:, tap, :],
                        srcp[:, y0 + dy : y0 + dy + 8, dx : dx + W],
                        start=(tap == 0),
                        stop=(tap == 8),
                    )
                st = s_pool.tile([C, 512], f32, tag="s")
                if c8 % 2 == 0:
                    nc.vector.tensor_copy(r(st[:]), ps[:])
                else:
                    nc.scalar.copy(r(st[:]), ps[:])
                s_t.append(st)

            # ---- forward QKV GEMM: q and v in channels-on-partitions layout ----
            vpad = vpad_pool.tile([C, H + 4, W + 4], fp8, tag="vpad")
            nc.gpsimd.memset(vpad[:, 0:2, :], 0.0)
            nc.gpsimd.memset(vpad[:, H + 2 : H + 4, :], 0.0)
            nc.gpsimd.memset(vpad[:, :, 0:2], 0.0)
            nc.gpsimd.memset(vpad[:, :, W + 2 : W + 4], 0.0)
            q_t = []
            for c8 in range(NC8):
                psq = ps_pool.tile([C, 512], f32, tag="ps")
                nc.tensor.matmul(psq[:], r(wq_sb[:]), r(s_t[c8][:]), start=True, stop=True)
                qt = q_pool.tile([C, 512], f32, tag="q")
                qeng = nc.scalar if c8 % 2 == 0 else nc.vector
                veng = nc.vector if c8 % 2 == 0 else nc.scalar
                if qeng is nc.scalar:
                    nc.scalar.copy(r(qt[:]), psq[:])
                else:
                    nc.vector.tensor_copy(r(qt[:]), psq[:])
                q_t.append(qt)
                psv = ps_pool.tile([C, 512], f32, tag="ps")
                nc.tensor.matmul(psv[:], r(wv_sb[:]), r(s_t[c8][:]), start=True, stop=True)
                vdst = vpad[:, 2 + 8 * c8 : 2 + 8 * c8 + 8, 2 : W + 2]
                vsrc = psv[:].rearrange("p (a b) -> p a b", a=8)
                if veng is nc.scalar:
                    nc.scalar.copy(vdst, vsrc)
                else:
                    nc.vector.tensor_copy(vdst, vsrc)
                if c8 % 2 == 1:
                    k = c8 // 2
                    nc.vector.reduce_sum(
                        pool_part[:, k : k + 1],
                        srcp[:, 1 + 16 * k : 1 + 16 * (k + 1), 1 : W + 1],
                        axis=mybir.AxisListType.XY,
                    )
                    if c8 == NC8 - 1:
                        nc.vector.reduce_sum(
                            pool_sum[:], pool_part[:], axis=mybir.AxisListType.X
                        )

            # ---- transposed GEMM: [kT | vT] chunks; exp(kT) -> eT, vT + ones col ----
            eT = eT_pool.tile([C, NJ, C], bf16, tag="eT")
            vT = vT_pool.tile([C, NJ, C + 1], bf16, tag="vT")
            nc.gpsimd.memset(vT[:, :, C : C + 1], 1.0)
            for j in range(NJ):
                psT = ps_pool.tile([C, 512], f32, tag="ps", name="psT")
                lhs = s_t[j // 4][:, (j % 4) * 128 : (j % 4 + 1) * 128]
                nc.tensor.matmul(
                    psT[:, 0 : 2 * C], r(lhs), r(wkv_sb[:]), start=True, stop=True
                )
                nc.scalar.activation(eT[:, j, :], psT[:, 0:C], Act.Exp)
                nc.vector.tensor_copy(vT[:, j, 0:C], psT[:, C : 2 * C])

            state.append(dict(q_t=q_t, vpad=vpad, eT=eT, vT=vT, pool_sum=pool_sum))

        # ---- pass 2: per-sample fused conv/content/out-proj pipeline. ----
        for smp in range(SPC):
            srcp = srcp_t[smp]
            q_t = state[smp]["q_t"]
            vpad = state[smp]["vpad"]
            eT = state[smp]["eT"]
            vT = state[smp]["vT"]
            pool_sum = state[smp]["pool_sum"]

            # ---- fused conv/content/out-proj pipeline over chunks.
            # ps5 runs one chunk ahead; the CL block is emitted after ps5(0)
            # so its eT/vT copy drain hides under conv matmuls; out-proj of
            # chunk c8 runs while conv of chunk c8+2 streams. Out-proj stages
            # one [C, 4, 512] tile per chunk and DMAs it in one transfer,
            # rotating engine queues; the last chunk splits across 4 queues
            # to keep the kernel tail short. ----
            def conv5(c8):
                ps5 = ps_pool.tile([C, 512], f32, tag="ps")
                y0 = 8 * c8
                for j in range(13):
                    ta, tb = 2 * j, min(2 * j + 1, 24)
                    dya, dxa = ta // 5, ta % 5
                    dyb, dxb = tb // 5, tb % 5
                    delta = (dyb - dya) * WROW + (dxb - dxa)
                    base = vpad[:, y0 + dya : y0 + dya + 8, dxa : dxa + W]
                    rhs = bass.AP(
                        base.tensor,
                        base.offset,
                        [list(base.ap[0]), [delta, 2], [WROW, 8], [1, W]],
                    )
                    nc.tensor.matmul(
                        ps5[:], d5_sb[:, j, :, :], rhs,
                        start=(j == 0), stop=(j == 12), perf_mode=DR,
                    )
                return ps5

            def content(c8, ps5):
                psc = ps_pool.tile([C, 512], f32, tag="ps")
                nc.tensor.matmul(psc[:], r(cln[0]), r(q_t[c8][:]), start=True, stop=True)
                tmp = tmp_pool.tile([C, 512], f32, tag="tmp")
                nc.vector.tensor_tensor(tmp[:], q_t[c8][:], ps5[:], Alu.mult)
                rt = r1_pool.tile([C, 512], f32, tag="r1")
                nc.vector.tensor_tensor(r(rt[:]), tmp[:], psc[:], Alu.add)
                return rt

            def outproj(c8, rt):
                y0 = 8 * c8
                # the last sample's final two chunks split per-m across all
                # three DMA queues to keep the kernel tail short
                split = smp == SPC - 1 and c8 >= NC8 - 2
                stg = stage_pool.tile([C, 4, 512], bf16, tag="stage")
                for m in range(OUT // C):
                    pso = ps_pool.tile([C, 512], f32, tag="ps")
                    nc.tensor.matmul(
                        pso[:], r(w1_sb[:, m * C : (m + 1) * C]), r(rt[:]),
                        start=True, stop=False,
                    )
                    nc.tensor.matmul(
                        pso[:], w2p[0][:, m * C : (m + 1) * C],
                        srcp[:, 1 + y0 : 1 + y0 + 8, 1 : W + 1],
                        start=False, stop=True,
                    )
                    if m % 2 == 0:
                        nc.scalar.copy(stg[:, m, :], pso[:])
                    else:
                        nc.vector.tensor_copy(stg[:, m, :], pso[:])
                    if split:
                        eng = (nc.gpsimd, nc.sync, nc.scalar)[(4 * c8 + m) % 3]
                        eng.dma_start(
                            out_v[smp, m * C : (m + 1) * C, c8 * 512 : (c8 + 1) * 512],
                            stg[:, m, :],
                        )
                if not split:
                    for h in range(2):
                        dst = bass.AP(
                            out_v.tensor,
                            out_v.offset + (smp * OUT + 2 * h * C) * N + c8 * 512,
                            [[N, C], [C * N, 2], [1, 512]],
                        )
                        eng = (nc.gpsimd, nc.sync, nc.scalar)[(2 * c8 + h) % 3]
                        eng.dma_start(dst, stg[:, 2 * h : 2 * h + 2, :])

            cln = [None]
            w2p = [None]
            pend5 = {}   # c8 -> ps5 awaiting content
            pendo = {}   # c8 -> rt awaiting out-projection
            for c8 in range(NC8):
                if c8 >= 1:
                    pendo[c8 - 1] = content(c8 - 1, pend5.pop(c8 - 1))
                if c8 >= 2:
                    outproj(c8 - 2, pendo.pop(c8 - 2))
                pend5[c8] = conv5(c8)
                if c8 == 0:
                    # ECA: ca = sigmoid(tridiag @ mean_pool(src)); w2 scaled
                    ps_eca = ps_pool.tile([C, 1], f32, tag="ps", name="ps_eca")
                    nc.tensor.matmul(
                        ps_eca[:], trid_sb[:], pool_sum[:], start=True, stop=True
                    )
                    ca = small_pool.tile([C, 1], f32, tag="ca")
                    nc.scalar.activation(ca[:], ps_eca[:], Act.Sigmoid)
                    w2p[0] = w2p_pool.tile([C, OUT], bf16, tag="w2p", name="w2p")
                    nc.vector.tensor_scalar(w2p[0][:], w2_sb[:], ca[:], None, Alu.mult)
                    # content lambda: CL[i, o] (+ row sums in col 128)
                    ps_cl = ps_pool.tile([C, C + 1], f32, tag="ps", name="ps_cl")
                    for j in range(NJ):
                        nc.tensor.matmul(
                            ps_cl[:], eT[:, j, :], vT[:, j, :],
                            start=(j == 0), stop=(j == NJ - 1),
                        )
                    recip = small_pool.tile([C, 1], f32, tag="recip")
                    nc.vector.reciprocal(recip[:], ps_cl[:, C : C + 1])
                    cln_t = small_pool.tile([C, C], f32, tag="cln_t")
                    nc.vector.tensor_scalar(
                        cln_t[:], ps_cl[:, 0:C], recip[:], None, Alu.mult
                    )
                    cln[0] = cln_pool.tile([C, C], f32, tag="cln", name="cln")
                    nc.vector.tensor_tensor(r(cln[0][:]), cln_t[:], mask_sb[:], Alu.mult)
            pendo[NC8 - 1] = content(NC8 - 1, pend5.pop(NC8 - 1))
            outproj(NC8 - 2, pendo.pop(NC8 - 2))
            outproj(NC8 - 1, pendo.pop(NC8 - 1))

    nc.compile()
    return nc


def _get_nc():
    if "nc" not in _CACHE:
        _CACHE["nc"] = _build_nc()
    return _CACHE["nc"]


def _host_weights(cpe_w, qkv_w, rel_pos, conv1d_w, out_w):
    cpe_w = np.asarray(cpe_w, np.float32)
    qkv_w = np.asarray(qkv_w, np.float32)
    rel_pos = np.asarray(rel_pos, np.float32)
    conv1d_w = np.asarray(conv1d_w, np.float32)
    out_w = np.asarray(out_w, np.float32)

    import ml_dtypes
    d3 = np.zeros([C, 9, C], ml_dtypes.bfloat16)
    idx = np.arange(C)
    for tap in range(9):
        dy, dx = tap // 3, tap % 3
        d3[idx, tap, idx] = cpe_w[:, 0, dy, dx].astype(ml_dtypes.bfloat16)
    d3[idx, 4, idx] = (cpe_w[:, 0, 1, 1].astype(np.float32) + 1.0).astype(
        ml_dtypes.bfloat16
    )  # residual folded into center tap

    # 5x5 rel-pos weights as fp8 DoubleRow pairs, pre-scaled by PLS so the
    # ~0.02-magnitude weights land in fp8e4m3's normal range. The position
    # path output is then PLS x too large; compensated by mask*PLS (content
    # path, so result1 is uniformly scaled) and w1/PLS (out projection).
    import ml_dtypes
    d5 = np.zeros([C, 13, 2, C], ml_dtypes.float8_e4m3fn)
    for tap in range(25):
        dy, dx = tap // 5, tap % 5
        d5[idx, tap // 2, tap % 2, idx] = (rel_pos[idx % HD, dy, dx] * PLS).astype(
            ml_dtypes.float8_e4m3fn
        )

    wq = np.ascontiguousarray(qkv_w[0:C, :].T)
    wv = np.ascontiguousarray(qkv_w[2 * C : 3 * C, :].T)
    wkv = np.ascontiguousarray(qkv_w[C : 3 * C, :].T)
    w1 = np.ascontiguousarray(out_w[:, 0:C].T) * (1.0 / PLS)
    w2 = np.ascontiguousarray(out_w[:, C : 2 * C].T)

    mask = np.zeros([C, C], np.float32)
    for h in range(NH):
        mask[h * HD : (h + 1) * HD, h * HD : (h + 1) * HD] = SCALING * PLS

    trid = np.zeros([C, C], np.float32)
    trid[idx[:-1], idx[:-1] + 1] = conv1d_w[0]  # pool[c-1] contributes to ca[c]
    trid[idx, idx] = conv1d_w[1]
    trid[idx[1:], idx[1:] - 1] = conv1d_w[2]
    trid *= 1.0 / N
    return dict(d3=d3, d5=d5, wq=wq, wv=wv, wkv=wkv, w1=w1, w2=w2,
                mask=mask, trid=trid)


def kernel(src, cpe_w, qkv_w, rel_pos, conv1d_w, out_w):
    from concourse.bass_utils import run_bass_kernel_spmd

    import ml_dtypes
    src = np.asarray(src, np.float32)
    src = np.pad(src, ((0, 0), (0, 0), (1, 1), (1, 1))).astype(ml_dtypes.bfloat16)
    w = _host_weights(cpe_w, qkv_w, rel_pos, conv1d_w, out_w)
    nc = _get_nc()
    in_maps = [
        {"src": np.ascontiguousarray(src[i * SPC : (i + 1) * SPC]), **w}
        for i in range(NCORES)
    ]
    trace = bool(os.environ.get("BASS_TRACE"))
    res = run_bass_kernel_spmd(nc, in_maps, list(range(NCORES)), trace=trace)
    _CACHE["last_result"] = res
    out = np.concatenate(
        [np.asarray(res.results[i]["out"], np.float32) for i in range(NCORES)], axis=0
    )
    return out


# revision 37
# speedup vs baseline: 1.0048x; 1.0014x over previous
# Bass/Trainium2 kernel for MHConvAttention (B=16, C=128, H=W=64, NH=8, OUT=512)
# Data-parallel over batch: 8 cores x 2 samples each.
#
# Per-sample layout: channels (128) on SBUF partitions, flattened spatial (4096)
# on the free dim. Depthwise convs are diagonal-weight matmuls accumulated in
# PSUM over zero-padded buffers; the 5x5 rel-pos conv runs as fp8 DoubleRow
# tap-pairs (2 taps per matmul, weights pre-scaled by PLS and folded back out
# via mask/w1); the content-lambda path uses a transposed QKV GEMM (spatial on
# partitions) so no explicit transposes are needed; the ECA channel-attention
# is folded into the out-projection weights. Inputs arrive host-padded in bf16
# so every DMA is contiguous; the output is written in bf16 (cast back to f32
# on the host) to halve output DMA bytes. Both samples' prep phases run before
# the two fused conv/content/out-proj chunk pipelines so the output DMA stream
# spreads across most of the kernel, rotating over the gpsimd/sync/scalar
# queues; the last two chunks split per-m to keep the kernel tail short.
import os
import numpy as np

B, C, H, W = 16, 128, 64, 64
NH, HD, WIN, OUT = 8, 16, 5, 512
N = H * W
NCORES = 8
SPC = B // NCORES          # samples per core
NC8 = N // 512             # 512-wide chunks per sample
NJ = N // 128              # 128-wide chunks (transposed GEMM)
SCALING = HD ** (-0.5)
PLS = 64.0                 # fp8 pre-scale for the 5x5 rel-pos weights

_CACHE = {}


def _build_nc():
    import concourse.bass as bass
    import concourse.tile as tile
    import concourse.mybir as mybir
    from concourse import bacc

    f32 = mybir.dt.float32
    f32r = mybir.dt.float32r
    bf16 = mybir.dt.bfloat16
    fp8 = mybir.dt.float8e4
    DR = mybir.MatmulPerfMode.DoubleRow
    Alu = mybir.AluOpType
    Act = mybir.ActivationFunctionType

    def r(ap):
        return ap.bitcast(f32r)

    nc = bacc.Bacc(trn_type="TRN2", target_bir_lowering=False, debug=False)

    src_d = nc.dram_tensor("src", [SPC, C, H + 2, W + 2], bf16, kind="ExternalInput").ap()
    d3_d = nc.dram_tensor("d3", [C, 9, C], bf16, kind="ExternalInput").ap()
    d5_d = nc.dram_tensor("d5", [C, 13, 2, C], fp8, kind="ExternalInput").ap()
    wq_d = nc.dram_tensor("wq", [C, C], f32, kind="ExternalInput").ap()
    wv_d = nc.dram_tensor("wv", [C, C], f32, kind="ExternalInput").ap()
    wkv_d = nc.dram_tensor("wkv", [C, 2 * C], f32, kind="ExternalInput").ap()
    w1_d = nc.dram_tensor("w1", [C, OUT], f32, kind="ExternalInput").ap()
    w2_d = nc.dram_tensor("w2", [C, OUT], f32, kind="ExternalInput").ap()
    mask_d = nc.dram_tensor("mask", [C, C], f32, kind="ExternalInput").ap()
    trid_d = nc.dram_tensor("trid", [C, C], f32, kind="ExternalInput").ap()
    out_d = nc.dram_tensor("out", [SPC, OUT, H, W], bf16, kind="ExternalOutput").ap()
    out_v = out_d.rearrange("s o h w -> s o (h w)")

    with tile.TileContext(nc) as tc, __import__("contextlib").ExitStack() as ctx:
        wpool = ctx.enter_context(tc.tile_pool(name="w", bufs=1))
        srcp_pool = ctx.enter_context(tc.tile_pool(name="srcp", bufs=2))
        s_pool = ctx.enter_context(tc.tile_pool(name="s", bufs=8))
        q_pool = ctx.enter_context(tc.tile_pool(name="q", bufs=16))
        r1_pool = ctx.enter_context(tc.tile_pool(name="r1", bufs=3))
        vpad_pool = ctx.enter_context(tc.tile_pool(name="vpad", bufs=2))
        eT_pool = ctx.enter_context(tc.tile_pool(name="eT", bufs=2))
        vT_pool = ctx.enter_context(tc.tile_pool(name="vT", bufs=2))
        tmp_pool = ctx.enter_context(tc.tile_pool(name="tmp", bufs=2))
        w2p_pool = ctx.enter_context(tc.tile_pool(name="w2p", bufs=2))
        stage_pool = ctx.enter_context(tc.tile_pool(name="stage", bufs=4))
        cln_pool = ctx.enter_context(tc.tile_pool(name="cln", bufs=2))
        small_pool = ctx.enter_context(tc.tile_pool(name="small", bufs=4))
        ps_pool = ctx.enter_context(tc.tile_pool(name="ps", bufs=8, space="PSUM"))

        # ---- prologue DMAs: weights on the scalar queue (in order of first
        # use), per-sample src slices on sync/gpsimd queues ----
        d3_sb = wpool.tile([C, 9, C], bf16)
        for sl in range(3):
            nc.scalar.dma_start(
                d3_sb[:, 3 * sl : 3 * sl + 3, :], d3_d[:, 3 * sl : 3 * sl + 3, :]
            )
        srcp_t = []
        rsplit = [0, 10, 18, 26, 34, 42, 50, 58, 66]
        for smp in range(SPC):
            srcp = srcp_pool.tile([C, H + 2, W + 2], bf16, tag="srcp")
            for k in range(8):
                lo, hi = rsplit[k], rsplit[k + 1]
                eng = nc.sync if k % 2 == 0 else nc.gpsimd
                eng.dma_start(
                    srcp[:, lo:hi, :],
                    src_d[smp, :, lo:hi, :],
                )
            srcp_t.append(srcp)
        wq_sb = wpool.tile([C, C], f32)
        nc.scalar.dma_start(r(wq_sb[:]), r(wq_d[:]))
        wv_sb = wpool.tile([C, C], f32)
        nc.scalar.dma_start(r(wv_sb[:]), r(wv_d[:]))
        wkv_sb = wpool.tile([C, 2 * C], f32)
        nc.scalar.dma_start(r(wkv_sb[:]), r(wkv_d[:]))
        d5_sb = wpool.tile([C, 13, 2, C], fp8)
        nc.scalar.dma_start(d5_sb[:], d5_d[:])
        trid_sb = wpool.tile([C, C], f32)
        nc.scalar.dma_start(trid_sb[:], trid_d[:])
        mask_sb = wpool.tile([C, C], f32)
        nc.scalar.dma_start(mask_sb[:], mask_d[:])
        w1_sb = wpool.tile([C, OUT], f32)
        nc.scalar.dma_start(r(w1_sb[:]), r(w1_d[:]))
        w2_sb = wpool.tile([C, OUT], f32)
        nc.scalar.dma_start(w2_sb[:], w2_d[:])

        WROW = W + 4
        state = []
        # ---- pass 1: both samples' CPE / forward QKV / transposed KV GEMMs.
        # Doing all prep first lets each sample's out-projection DMA stream
        # start earlier and spread over a longer window. ----
        for smp in range(SPC):
            srcp = srcp_t[smp]

            # ---- ECA pooling: 4 partial reduces interleaved with QKV copies ----
            pool_part = small_pool.tile([C, 4], f32, tag="pool_part")
            pool_sum = small_pool.tile([C, 1], f32, tag="psum_vec")

            # ---- CPE 3x3 depthwise conv (+residual, folded): s = conv3(src) ----
            s_t = []
            for c8 in range(NC8):
                ps = ps_pool.tile([C, 512], f32, tag="ps")
                y0 = 8 * c8
                for tap in range(9):
                    dy, dx = tap // 3, tap % 3
                    nc.tensor.matmul(
                        ps[:],
                        d3_sb[:, tap, :],
                        srcp[:, y0 + dy : y0 + dy + 8, dx : dx + W],
                        start=(tap == 0),
                        stop=(tap == 8),
                    )
                st = s_pool.tile([C, 512], f32, tag="s")
                if c8 % 2 == 0:
                    nc.vector.tensor_copy(r(st[:]), ps[:])
                else:
                    nc.scalar.copy(r(st[:]), ps[:])
                s_t.append(st)

            # ---- forward QKV GEMM: q and v in channels-on-partitions layout ----
            vpad = vpad_pool.tile([C, H + 4, W + 4], fp8, tag="vpad")
            nc.gpsimd.memset(vpad[:, 0:2, :], 0.0)
            nc.gpsimd.memset(vpad[:, H + 2 : H + 4, :], 0.0)
            nc.gpsimd.memset(vpad[:, :, 0:2], 0.0)
            nc.gpsimd.memset(vpad[:, :, W + 2 : W + 4], 0.0)
            q_t = []
            for c8 in range(NC8):
                psq = ps_pool.tile([C, 512], f32, tag="ps")
                nc.tensor.matmul(psq[:], r(wq_sb[:]), r(s_t[c8][:]), start=True, stop=True)
                qt = q_pool.tile([C, 512], f32, tag="q")
                qeng = nc.scalar if c8 % 2 == 0 else nc.vector
                veng = nc.vector if c8 % 2 == 0 else nc.scalar
                if qeng is nc.scalar:
                    nc.scalar.copy(r(qt[:]), psq[:])
                else:
                    nc.vector.tensor_copy(r(qt[:]), psq[:])
                q_t.append(qt)
                psv = ps_pool.tile([C, 512], f32, tag="ps")
                nc.tensor.matmul(psv[:], r(wv_sb[:]), r(s_t[c8][:]), start=True, stop=True)
                vdst = vpad[:, 2 + 8 * c8 : 2 + 8 * c8 + 8, 2 : W + 2]
                vsrc = psv[:].rearrange("p (a b) -> p a b", a=8)
                if veng is nc.scalar:
                    nc.scalar.copy(vdst, vsrc)
                else:
                    nc.vector.tensor_copy(vdst, vsrc)
                if c8 % 2 == 1:
                    k = c8 // 2
                    nc.vector.reduce_sum(
                        pool_part[:, k : k + 1],
                        srcp[:, 1 + 16 * k : 1 + 16 * (k + 1), 1 : W + 1],
                        axis=mybir.AxisListType.XY,
                    )
                    if c8 == NC8 - 1:
                        nc.vector.reduce_sum(
                            pool_sum[:], pool_part[:], axis=mybir.AxisListType.X
                        )

            # ---- transposed GEMM: [kT | vT] chunks; exp(kT) -> eT, vT + ones col ----
            eT = eT_pool.tile([C, NJ, C], bf16, tag="eT")
            vT = vT_pool.tile([C, NJ, C + 1], bf16, tag="vT")
            nc.gpsimd.memset(vT[:, :, C : C + 1], 1.0)
            for j in range(NJ):
                psT = ps_pool.tile([C, 512], f32, tag="ps", name="psT")
                lhs = s_t[j // 4][:, (j % 4) * 128 : (j % 4 + 1) * 128]
                nc.tensor.matmul(
                    psT[:, 0 : 2 * C], r(lhs), r(wkv_sb[:]), start=True, stop=True
                )
                nc.scalar.activation(eT[:, j, :], psT[:, 0:C], Act.Exp)
                nc.vector.tensor_copy(vT[:, j, 0:C], psT[:, C : 2 * C])

            state.append(dict(q_t=q_t, vpad=vpad, eT=eT, vT=vT, pool_sum=pool_sum))

        # ---- pass 2: per-sample fused conv/content/out-proj pipeline. ----
        for smp in range(SPC):
            srcp = srcp_t[smp]
            q_t = state[smp]["q_t"]
            vpad = state[smp]["vpad"]
            eT = state[smp]["eT"]
            vT = state[smp]["vT"]
            pool_sum = state[smp]["pool_sum"]

            # ---- fused conv/content/out-proj pipeline over chunks.
            # ps5 runs one chunk ahead; the CL block is emitted after ps5(0)
            # so its eT/vT copy drain hides under conv matmuls; out-proj of
            # chunk c8 runs while conv of chunk c8+2 streams. Out-proj stages
            # one [C, 4, 512] tile per chunk and DMAs it in one transfer,
            # rotating engine queues; the last chunk splits across 4 queues
            # to keep the kernel tail short. ----
            def conv5(c8):
                # 12 fp8 DoubleRow pairs (taps 0-23) + tap 24 as a plain fp8
                # matmul: a zero-padded 13th pair would stream 512 wasted rows.
                ps5 = ps_pool.tile([C, 512], f32, tag="ps")
                y0 = 8 * c8
                for j in range(12):
                    ta, tb = 2 * j, 2 * j + 1
                    dya, dxa = ta // 5, ta % 5
                    dyb, dxb = tb // 5, tb % 5
                    delta = (dyb - dya) * WROW + (dxb - dxa)
                    base = vpad[:, y0 + dya : y0 + dya + 8, dxa : dxa + W]
                    rhs = bass.AP(
                        base.tensor,
                        base.offset,
                        [list(base.ap[0]), [delta, 2], [WROW, 8], [1, W]],
                    )
                    nc.tensor.matmul(
                        ps5[:], d5_sb[:, j, :, :], rhs,
                        start=(j == 0), stop=False, perf_mode=DR,
                    )
                nc.tensor.matmul(
                    ps5[:], d5_sb[:, 12, 0, :],
                    vpad[:, y0 + 4 : y0 + 4 + 8, 4 : 4 + W],
                    start=False, stop=True,
                )
                return ps5

            def content(c8, ps5):
                psc = ps_pool.tile([C, 512], f32, tag="ps")
                nc.tensor.matmul(psc[:], r(cln[0]), r(q_t[c8][:]), start=True, stop=True)
                tmp = tmp_pool.tile([C, 512], f32, tag="tmp")
                nc.vector.tensor_tensor(tmp[:], q_t[c8][:], ps5[:], Alu.mult)
                rt = r1_pool.tile([C, 512], f32, tag="r1")
                nc.vector.tensor_tensor(r(rt[:]), tmp[:], psc[:], Alu.add)
                return rt

            def outproj(c8, rt):
                y0 = 8 * c8
                # the last sample's final two chunks split per-m across all
                # three DMA queues to keep the kernel tail short
                split = smp == SPC - 1 and c8 >= NC8 - 2
                stg = stage_pool.tile([C, 4, 512], bf16, tag="stage")
                for m in range(OUT // C):
                    pso = ps_pool.tile([C, 512], f32, tag="ps")
                    nc.tensor.matmul(
                        pso[:], r(w1_sb[:, m * C : (m + 1) * C]), r(rt[:]),
                        start=True, stop=False,
                    )
                    nc.tensor.matmul(
                        pso[:], w2p[0][:, m * C : (m + 1) * C],
                        srcp[:, 1 + y0 : 1 + y0 + 8, 1 : W + 1],
                        start=False, stop=True,
                    )
                    if m % 2 == 0:
                        nc.scalar.copy(stg[:, m, :], pso[:])
                    else:
                        nc.vector.tensor_copy(stg[:, m, :], pso[:])
                    if split:
                        eng = (nc.gpsimd, nc.sync, nc.scalar)[(4 * c8 + m) % 3]
                        eng.dma_start(
                            out_v[smp, m * C : (m + 1) * C, c8 * 512 : (c8 + 1) * 512],
                            stg[:, m, :],
                        )
                if not split:
                    for h in range(2):
                        dst = bass.AP(
                            out_v.tensor,
                            out_v.offset + (smp * OUT + 2 * h * C) * N + c8 * 512,
                            [[N, C], [C * N, 2], [1, 512]],
                        )
                        eng = (nc.gpsimd, nc.sync, nc.scalar)[(2 * c8 + h) % 3]
                        eng.dma_start(dst, stg[:, 2 * h : 2 * h + 2, :])

            cln = [None]
            w2p = [None]
            pend5 = {}   # c8 -> ps5 awaiting content
            pendo = {}   # c8 -> rt awaiting out-projection
            for c8 in range(NC8):
                if c8 >= 1:
                    pendo[c8 - 1] = content(c8 - 1, pend5.pop(c8 - 1))
                if c8 >= 2:
                    outproj(c8 - 2, pendo.pop(c8 - 2))
                pend5[c8] = conv5(c8)
                if c8 == 0:
                    # ECA: ca = sigmoid(tridiag @ mean_pool(src)); w2 scaled
                    ps_eca = ps_pool.tile([C, 1], f32, tag="ps", name="ps_eca")
                    nc.tensor.matmul(
                        ps_eca[:], trid_sb[:], pool_sum[:], start=True, stop=True
                    )
                    ca = small_pool.tile([C, 1], f32, tag="ca")
                    nc.scalar.activation(ca[:], ps_eca[:], Act.Sigmoid)
                    w2p[0] = w2p_pool.tile([C, OUT], bf16, tag="w2p", name="w2p")
                    nc.vector.tensor_scalar(w2p[0][:], w2_sb[:], ca[:], None, Alu.mult)
                    # content lambda: CL[i, o] (+ row sums in col 128)
                    ps_cl = ps_pool.tile([C, C + 1], f32, tag="ps", name="ps_cl")
                    for j in range(NJ):
                        nc.tensor.matmul(
                            ps_cl[:], eT[:, j, :], vT[:, j, :],
                            start=(j == 0), stop=(j == NJ - 1),
                        )
                    recip = small_pool.tile([C, 1], f32, tag="recip")
                    nc.vector.reciprocal(recip[:], ps_cl[:, C : C + 1])
                    cln_t = small_pool.tile([C, C], f32, tag="cln_t")
                    nc.vector.tensor_scalar(
                        cln_t[:], ps_cl[:, 0:C], recip[:], None, Alu.mult
                    )
                    cln[0] = cln_pool.tile([C, C], f32, tag="cln", name="cln")
                    nc.vector.tensor_tensor(r(cln[0][:]), cln_t[:], mask_sb[:], Alu.mult)
            pendo[NC8 - 1] = content(NC8 - 1, pend5.pop(NC8 - 1))
            outproj(NC8 - 2, pendo.pop(NC8 - 2))
            outproj(NC8 - 1, pendo.pop(NC8 - 1))

    nc.compile()
    return nc


def _get_nc():
    if "nc" not in _CACHE:
        _CACHE["nc"] = _build_nc()
    return _CACHE["nc"]


def _host_weights(cpe_w, qkv_w, rel_pos, conv1d_w, out_w):
    cpe_w = np.asarray(cpe_w, np.float32)
    qkv_w = np.asarray(qkv_w, np.float32)
    rel_pos = np.asarray(rel_pos, np.float32)
    conv1d_w = np.asarray(conv1d_w, np.float32)
    out_w = np.asarray(out_w, np.float32)

    import ml_dtypes
    d3 = np.zeros([C, 9, C], ml_dtypes.bfloat16)
    idx = np.arange(C)
    for tap in range(9):
        dy, dx = tap // 3, tap % 3
        d3[idx, tap, idx] = cpe_w[:, 0, dy, dx].astype(ml_dtypes.bfloat16)
    d3[idx, 4, idx] = (cpe_w[:, 0, 1, 1].astype(np.float32) + 1.0).astype(
        ml_dtypes.bfloat16
    )  # residual folded into center tap

    # 5x5 rel-pos weights as fp8 DoubleRow pairs, pre-scaled by PLS so the
    # ~0.02-magnitude weights land in fp8e4m3's normal range. The position
    # path output is then PLS x too large; compensated by mask*PLS (content
    # path, so result1 is uniformly scaled) and w1/PLS (out projection).
    import ml_dtypes
    d5 = np.zeros([C, 13, 2, C], ml_dtypes.float8_e4m3fn)
    for tap in range(25):
        dy, dx = tap // 5, tap % 5
        d5[idx, tap // 2, tap % 2, idx] = (rel_pos[idx % HD, dy, dx] * PLS).astype(
            ml_dtypes.float8_e4m3fn
        )

    wq = np.ascontiguousarray(qkv_w[0:C, :].T)
    wv = np.ascontiguousarray(qkv_w[2 * C : 3 * C, :].T)
    wkv = np.ascontiguousarray(qkv_w[C : 3 * C, :].T)
    w1 = np.ascontiguousarray(out_w[:, 0:C].T) * (1.0 / PLS)
    w2 = np.ascontiguousarray(out_w[:, C : 2 * C].T)

    mask = np.zeros([C, C], np.float32)
    for h in range(NH):
        mask[h * HD : (h + 1) * HD, h * HD : (h + 1) * HD] = SCALING * PLS

    trid = np.zeros([C, C], np.float32)
    trid[idx[:-1], idx[:-1] + 1] = conv1d_w[0]  # pool[c-1] contributes to ca[c]
    trid[idx, idx] = conv1d_w[1]
    trid[idx[1:], idx[1:] - 1] = conv1d_w[2]
    trid *= 1.0 / N
    return dict(d3=d3, d5=d5, wq=wq, wv=wv, wkv=wkv, w1=w1, w2=w2,
                mask=mask, trid=trid)


def kernel(src, cpe_w, qkv_w, rel_pos, conv1d_w, out_w):
    from concourse.bass_utils import run_bass_kernel_spmd

    import ml_dtypes
    src = np.asarray(src, np.float32)
    src = np.pad(src, ((0, 0), (0, 0), (1, 1), (1, 1))).astype(ml_dtypes.bfloat16)
    w = _host_weights(cpe_w, qkv_w, rel_pos, conv1d_w, out_w)
    nc = _get_nc()
    in_maps = [
        {"src": np.ascontiguousarray(src[i * SPC : (i + 1) * SPC]), **w}
        for i in range(NCORES)
    ]
    trace = bool(os.environ.get("BASS_TRACE"))
    res = run_bass_kernel_spmd(nc, in_maps, list(range(NCORES)), trace=trace)
    _CACHE["last_result"] = res
    out = np.concatenate(
        [np.asarray(res.results[i]["out"], np.float32) for i in range(NCORES)], axis=0
    )
    return out
